# revision 5
# baseline (speedup 1.0000x reference)
"""Trainium2 Bass kernel for the CSA (channel-spatial attention) module.

Reference computation (per batch b):
    q = Wq @ x[b]            # [64, N]
    k = Wk @ x[b]            # [64, N]
    E[n, m] = sum_c q[c, n] * k[c, m]          # [N, N]
    A = softmax(E, axis=m)
    v = Wv @ x_h[b]          # [128, N]
    out[c, n] = sum_m v[c, m] * A[n, m]
    result = gamma * out + x_h[b]

Sharding: 8 cores = 4 batches x 2 query-halves. Each core holds full K/V for
its batch and a 2048-wide query chunk (flash-style: the [N, N] attention
matrix is never materialized in HBM).

Key transformations vs the naive mapping:
- Wk is folded into the query projection on the host:
  E^T[m, n] = sum_c' xb[c', m] * qk[c', n]  with  qk = (Wk^T Wq) @ x_chunk.
  aT is host-padded to [64, 128] so the projection matmul itself writes
  the zero rows 64..127 of qk (no memset on the critical path).
- Energy is computed transposed, E^T[m, n] (m on partitions), so
  exp(E^T) tiles feed the second matmul U[c, n] += vT.T @ P^T directly
  (PSUM-accumulated over m).
- V path: v = Wv @ x_h computed with 8 wide matmuls (wvT stationary),
  cast to bf16 on DVE, then transposed to vT via the DMA XBAR
  (dma_start_transpose, [128,512] -> 4 adjacent [128,128] blocks), which
  keeps 24 matmuls + transposes off the PE during the PE-heavy first
  group.
- Softmax denominator S[n]: exp tiles are accumulated in bf16 on the DVE
  (two 8-pair chains per group, full 1024-wide adds), then partition-
  reduced by matmuls against an ALL-ONES [128,128] stationary, which
  lands S replicated across all 128 PSUM partitions.  The reciprocal and
  the U*(1/S) multiply then run full-width on DVE with no broadcast
  matmul (a K=1 broadcast matmul also triggers HAM half-array throttle).
- GpSimd does only memsets + bulk DMA issue: its tensor ops contend with
  DVE for SBUF ports and slow both engines down.
- The E matmuls run 2 iterations ahead of the exp/U consumers; each
  group's epilogue PE work is deferred into the next group so the
  in-order PE queue never waits on DVE.
- bf16 operands throughout the attention math (fp32 PSUM accumulation,
  fp32 residual add).
- No max-subtraction: logits are N(0, 64), |E| << 88 (fp32 exp overflow).
"""

import numpy as np

import concourse.bass as bass
import concourse.mybir as mybir
import concourse.tile as tile
from concourse import bacc
from concourse.bass_utils import run_bass_kernel_spmd

B = 4
CQK = 64
CV = 128
N = 4096
NQ = N // 2          # query columns per core
NG = 512             # n-group width (PSUM bank)
MT = 128             # m-tile height (PE contraction tile)
N_GROUPS = NQ // NG  # 4
N_MTILES = N // MT   # 32

F32 = mybir.dt.float32
BF16 = mybir.dt.bfloat16


_last_results = None  # stashed BassKernelResults for test harnesses


def build_bass(gamma: float) -> bass.Bass:
    nc = bacc.Bacc()

    # xb rows CQK..127 are zero-padded on the host (full-K matmuls).
    xb = nc.declare_dram_parameter("xb", [MT, N], BF16, isOutput=False)
    xhb = nc.declare_dram_parameter("xhb", [CV, N], BF16, isOutput=False)
    xq = nc.declare_dram_parameter("xq", [CQK, NQ], BF16, isOutput=False)
    xh_res = nc.declare_dram_parameter("xh_res", [CV, NQ], F32, isOutput=False)
    aT = nc.declare_dram_parameter("aT", [CQK, MT], BF16, isOutput=False)
    wvT = nc.declare_dram_parameter("wvT", [CV, CV], BF16, isOutput=False)
    o = nc.declare_dram_parameter("o", [CV, NQ], F32, isOutput=True)

    ts = bass.ts

    with tile.TileContext(nc) as tc:
        with (
            nc.allow_low_precision(reason="bf16 attention math, fp32 accum"),
            tc.tile_pool(name="const", bufs=1) as cpool,
            tc.tile_pool(name="pt", bufs=4) as ptpool,
            tc.tile_pool(name="sacc", bufs=2) as sapool,
            tc.tile_pool(name="ep", bufs=2, space="PSUM") as epool,
            tc.tile_pool(name="up", bufs=2, space="PSUM") as upool,
            tc.tile_pool(name="sp", bufs=1, space="PSUM") as spool,
            tc.tile_pool(name="mp", bufs=1, space="PSUM") as mpool,
            tc.tile_pool(name="out", bufs=3) as opool,
        ):
            # ---- persistent SBUF tensors ----
            xb_sb = cpool.tile([MT, N], BF16)
            xhb_sb = cpool.tile([CV, N], BF16)
            xq_sb = cpool.tile([CQK, NQ], BF16)
            xhres_sb = cpool.tile([CV, NQ], F32)
            aT_sb = cpool.tile([CQK, MT], BF16)
            wvT_sb = cpool.tile([CV, CV], BF16)
            qk_sb = cpool.tile([MT, NQ], BF16)  # rows CQK..127 zero via aT pad
            v_sb = cpool.tile([CV, N], BF16)    # v = Wv @ x_h (c on partitions)
            vT_sb = cpool.tile([CV, N], BF16)   # cols [mt*128,(mt+1)*128) = v[:, chunk].T
            ones_g = cpool.tile([MT, MT], BF16)  # all-ones (S-reduce stationary)
            zbias = cpool.tile([MT, 1], F32)

            # ---- loads: critical-path tensors first, bulk on gpsimd queue ----
            nc.sync.dma_start(aT_sb[:], aT[:])
            nc.sync.dma_start(xq_sb[:, :NG], xq[:, :NG])
            nc.sync.dma_start(xq_sb[:, NG:], xq[:, NG:])
            for j in range(N // NG):
                nc.sync.dma_start(xb_sb[:, ts(j, NG)], xb[:, ts(j, NG)])

            nc.gpsimd.memset(zbias[:], 0.0)
            ones_stage = cpool.tile([MT, MT], F32)
            nc.gpsimd.memset(ones_stage[:], 1.0)
            nc.gpsimd.dma_start(wvT_sb[:], wvT[:])
            for j in range(N // NG):
                nc.gpsimd.dma_start(xhb_sb[:, ts(j, NG)], xhb[:, ts(j, NG)])
            for j in range(NQ // NG):
                nc.gpsimd.dma_start(xhres_sb[:, ts(j, NG)], xh_res[:, ts(j, NG)])
            nc.vector.tensor_copy(ones_g[:], ones_stage[:])

            # ---- qk projection chunk: qk[:, j*512:] = (Wk^T Wq) @ xq chunk ----
            # aT's zero columns 64..127 write qk's zero rows; scalar engine
            # (idle at startup) does the PSUM->SBUF bf16 copy.
            def emit_qk(j):
                qk_ps = epool.tile([MT, NG], F32, tag="e", name=f"qkp_{j}")
                nc.tensor.matmul(qk_ps[:], aT_sb[:], xq_sb[:, ts(j, NG)],
                                 start=True, stop=True)
                nc.scalar.copy(qk_sb[:, ts(j, NG)], qk_ps[:])

            # ---- v projection chunk j + XBAR transpose into vT ----
            def emit_vchunk(j):
                v_ps = mpool.tile([CV, NG], F32, tag="mpsum", name=f"vp_{j}")
                nc.tensor.matmul(v_ps[:], wvT_sb[:], xhb_sb[:, ts(j, NG)],
                                 start=True, stop=True)
                nc.vector.tensor_copy(v_sb[:, ts(j, NG)], v_ps[:])
                vt_dst = vT_sb[:, ts(j, NG)].rearrange(
                    "p (a b) -> p a b", a=NG // MT)
                nc.sync.dma_start_transpose(vt_dst, v_sb[:, ts(j, NG)])

            # ---- main flash loop (flat, software-pipelined, PAIRED) ----
            PIPE = 2          # pipeline depth in pairs
            NPAIRS_G = N_MTILES // 2
            NPT = N_GROUPS * NPAIRS_G
            HALF = NPAIRS_G // 2  # sub-chain length in pairs

            def emit_Epair(g, pp):
                e2 = epool.tile([MT, 2 * NG], F32, tag="e", name=f"e_{g}_{pp}")
                nc.tensor.matmul(e2[:, :NG], xb_sb[:, ts(2 * pp, MT)],
                                 qk_sb[:, ts(g, NG)], start=True, stop=True)
                nc.tensor.matmul(e2[:, NG:], xb_sb[:, ts(2 * pp + 1, MT)],
                                 qk_sb[:, ts(g, NG)], start=True, stop=True)
                return e2

            def emit_sreduce(s_ps, chain, first, last):
                nc.tensor.matmul(s_ps[:], ones_g[:], chain[:, :NG],
                                 start=first, stop=False)
                nc.tensor.matmul(s_ps[:], ones_g[:], chain[:, NG:],
                                 start=False, stop=last)

            def emit_epilogue(g, u_ps, s_ps, chainB, gamma):
                emit_sreduce(s_ps, chainB, first=False, last=True)
                r_sb = opool.tile([CV, NG], F32, tag="r", name=f"r_{g}")
                nc.vector.reciprocal_approx_fast(out=r_sb[:], in_=s_ps[:])
                o_sb = opool.tile([CV, NG], F32, tag="o", name=f"o_{g}")
                nc.vector.tensor_mul(o_sb[:], u_ps[:], r_sb[:])
                nc.vector.scalar_tensor_tensor(
                    out=o_sb[:], in0=o_sb[:], scalar=gamma,
                    in1=xhres_sb[:, ts(g, NG)],
                    op0=mybir.AluOpType.mult, op1=mybir.AluOpType.add)
                nc.sync.dma_start(o[:, ts(g, NG)], o_sb[:])

            # startup: qk chunk 0, then the first E-pairs immediately; the
            # remaining qk chunks and the v chunks interleave into group 0.
            emit_qk(0)
            e_tiles = {p: emit_Epair(p // NPAIRS_G, p % NPAIRS_G)
                       for p in range(PIPE)}
            emit_qk(1)
            u_ps = None
            s_ps = None
            chains = None
            pending = None
            for p in range(NPT):
                g, pp = divmod(p, NPAIRS_G)
                if pp == 0:
                    u_ps = upool.tile([CV, NG], F32, tag="u", name=f"u_{g}")
                    s_ps = spool.tile([CV, NG], F32, tag="s", name=f"s_{g}")
                    chains = [sapool.tile([MT, 2 * NG], BF16, tag=f"sc{c}",
                                          name=f"sc{c}_{g}")
                              for c in range(2)]
                pt2 = ptpool.tile([MT, 2 * NG], BF16, tag="pt",
                                  name=f"pt_{g}_{pp}")
                nc.scalar.activation(pt2[:], e_tiles.pop(p)[:],
                                     mybir.ActivationFunctionType.Exp,
                                     bias=zbias[:])
                if p + PIPE < NPT:
                    gn, ppn = divmod(p + PIPE, NPAIRS_G)
                    e_tiles[p + PIPE] = emit_Epair(gn, ppn)
                if g == 0:
                    if pp == 0:
                        emit_qk(2)
                        emit_vchunk(0)
                    elif pp == 1:
                        emit_qk(3)
                        emit_vchunk(1)
                    elif 2 <= pp <= 7:
                        emit_vchunk(pp)
                lastp = pp == NPAIRS_G - 1
                # U[c, n] += vT_tile.T @ P^T  (both halves of the pair)
                nc.tensor.matmul(u_ps[:], vT_sb[:, ts(2 * pp, MT)],
                                 pt2[:, :NG], start=(pp == 0), stop=False)
                nc.tensor.matmul(u_ps[:], vT_sb[:, ts(2 * pp + 1, MT)],
                                 pt2[:, NG:], start=False, stop=lastp)
                # S chain accumulation on DVE (bf16, full 1024-wide)
                sub = pp // HALF
                chain = chains[sub]
                if pp % HALF == 0:
                    nc.vector.tensor_copy(chain[:], pt2[:])
                else:
                    nc.vector.tensor_add(chain[:], chain[:], pt2[:])
                if pp == HALF + 2:
                    emit_sreduce(s_ps, chains[0], first=True, last=False)
                if pending is not None and (pp >= 5 or p == NPT - 1):
                    emit_epilogue(*pending)
                    pending = None
                if lastp:
                    pending = (g, u_ps, s_ps, chains[1], gamma)
            emit_epilogue(*pending)

    nc.compile()
    return nc


def kernel(x, x_h, Wq, Wk, Wv, gamma):
    global _last_results
    import ml_dtypes
    bf16 = ml_dtypes.bfloat16

    x = np.ascontiguousarray(np.asarray(x, dtype=np.float32))
    x_h = np.ascontiguousarray(np.asarray(x_h, dtype=np.float32))
    Wq = np.asarray(Wq, dtype=np.float32)
    Wk = np.asarray(Wk, dtype=np.float32)
    Wv = np.asarray(Wv, dtype=np.float32)
    gval = float(np.asarray(gamma).reshape(-1)[0])

    nc = build_bass(gval)

    # qk = (Wk^T Wq) @ xq  ->  stationary operand is (Wk^T Wq)^T = Wq^T Wk,
    # padded with zero columns so the matmul writes qk rows 64..127 as zero.
    aT = np.zeros((CQK, MT), dtype=np.float32)
    aT[:, :CQK] = Wq.T @ Wk
    aT = aT.astype(bf16)
    wvT = np.ascontiguousarray(Wv.T).astype(bf16)
    x_bf = x.astype(bf16)
    xb_pad = np.zeros((B, MT, N), dtype=bf16)
    xb_pad[:, :CQK, :] = x_bf

    in_maps = []
    for core in range(8):
        b, h = core // 2, core % 2
        sl = slice(h * NQ, (h + 1) * NQ)
        in_maps.append({
            "xb": xb_pad[b],
            "xhb": x_h[b].astype(bf16),
            "xq": np.ascontiguousarray(x_bf[b][:, sl]),
            "xh_res": np.ascontiguousarray(x_h[b][:, sl]),
            "aT": aT,
            "wvT": wvT,
        })

    res = run_bass_kernel_spmd(nc, in_maps, list(range(8)))
    _last_results = res

    out = np.empty((B, CV, N), dtype=np.float32)
    for core in range(8):
        b, h = core // 2, core % 2
        out[b][:, h * NQ:(h + 1) * NQ] = res.results[core]["o"]
    return out


# revision 10
# speedup vs baseline: 1.1274x; 1.1274x over previous
"""Trainium2 Bass kernel for the CSA (channel-spatial attention) module.

Reference computation (per batch b):
    q = Wq @ x[b]            # [64, N]
    k = Wk @ x[b]            # [64, N]
    E[n, m] = sum_c q[c, n] * k[c, m]          # [N, N]
    A = softmax(E, axis=m)
    v = Wv @ x_h[b]          # [128, N]
    out[c, n] = sum_m v[c, m] * A[n, m]
    result = gamma * out + x_h[b]

Sharding: 8 cores = 4 batches x 2 query-halves. Each core holds full K/V for
its batch and a 2048-wide query chunk (flash-style: the [N, N] attention
matrix is never materialized in HBM).

Key transformations vs the naive mapping:
- Wk is folded into the query projection on the host:
  E^T[m, n] = sum_c' xb[c', m] * qk[c', n]  with  qk = (Wk^T Wq) @ x_chunk.
  aT is host-padded to [64, 128] so the projection matmul itself writes
  the zero rows 64..127 of qk (no memset on the critical path).
- Energy is computed transposed, E^T[m, n] (m on partitions), so
  exp(E^T) tiles feed the second matmul U[c, n] += vT.T @ P^T directly
  (PSUM-accumulated over m).
- V path: v = Wv @ x_h computed with 8 wide matmuls (wvT stationary),
  cast to bf16 on DVE, then transposed to vT via the DMA XBAR
  (dma_start_transpose, [128,512] -> 4 adjacent [128,128] blocks), which
  keeps 24 matmuls + transposes off the PE during the PE-heavy first
  group.
- Softmax denominator S[n]: exp tiles are accumulated in bf16 on the DVE
  (two 8-pair chains per group, full 1024-wide adds), then partition-
  reduced by matmuls against an ALL-ONES [128,128] stationary, which
  lands S replicated across all 128 PSUM partitions.  The reciprocal and
  the U*(1/S) multiply then run full-width on DVE with no broadcast
  matmul (a K=1 broadcast matmul also triggers HAM half-array throttle).
- GpSimd does only memsets + bulk DMA issue: its tensor ops contend with
  DVE for SBUF ports and slow both engines down.
- The E matmuls run 2 iterations ahead of the exp/U consumers; each
  group's epilogue PE work is deferred into the next group so the
  in-order PE queue never waits on DVE.
- bf16 operands throughout the attention math (fp32 PSUM accumulation,
  fp32 residual add).
- No max-subtraction: logits are N(0, 64), |E| << 88 (fp32 exp overflow).
"""

import numpy as np

import concourse.bass as bass
import concourse.mybir as mybir
import concourse.tile as tile
from concourse import bacc
from concourse.bass_utils import run_bass_kernel_spmd

B = 4
CQK = 64
CV = 128
N = 4096
NQ = N // 2          # query columns per core
NG = 512             # n-group width (PSUM bank)
MT = 128             # m-tile height (PE contraction tile)
N_GROUPS = NQ // NG  # 4
N_MTILES = N // MT   # 32

F32 = mybir.dt.float32
BF16 = mybir.dt.bfloat16


_last_results = None  # stashed BassKernelResults for test harnesses


def build_bass(gamma: float) -> bass.Bass:
    nc = bacc.Bacc()

    # xb rows CQK..127 are zero-padded on the host (full-K matmuls).
    xb = nc.declare_dram_parameter("xb", [MT, N], BF16, isOutput=False)
    xhb = nc.declare_dram_parameter("xhb", [CV, N], BF16, isOutput=False)
    xq = nc.declare_dram_parameter("xq", [CQK, NQ], BF16, isOutput=False)
    xh_res = nc.declare_dram_parameter("xh_res", [CV, NQ], F32, isOutput=False)
    aT = nc.declare_dram_parameter("aT", [CQK, MT], BF16, isOutput=False)
    wvT = nc.declare_dram_parameter("wvT", [CV, CV], BF16, isOutput=False)
    o = nc.declare_dram_parameter("o", [CV, NQ], F32, isOutput=True)

    ts = bass.ts

    with tile.TileContext(nc) as tc:
        with (
            nc.allow_low_precision(reason="bf16 attention math, fp32 accum"),
            tc.tile_pool(name="const", bufs=1) as cpool,
            tc.tile_pool(name="pt", bufs=4) as ptpool,
            tc.tile_pool(name="sacc", bufs=2) as sapool,
            tc.tile_pool(name="ep", bufs=2, space="PSUM") as epool,
            tc.tile_pool(name="up", bufs=2, space="PSUM") as upool,
            tc.tile_pool(name="sp", bufs=1, space="PSUM") as spool,
            tc.tile_pool(name="mp", bufs=1, space="PSUM") as mpool,
            tc.tile_pool(name="out", bufs=3) as opool,
        ):
            # ---- persistent SBUF tensors ----
            xb_sb = cpool.tile([MT, N], BF16)
            xhb_sb = cpool.tile([CV, N], BF16)
            xq_sb = cpool.tile([CQK, NQ], BF16)
            xhres_sb = cpool.tile([CV, NQ], F32)
            aT_sb = cpool.tile([CQK, MT], BF16)
            wvT_sb = cpool.tile([CV, CV], BF16)
            qk_sb = cpool.tile([MT, NQ], BF16)  # rows CQK..127 zero via aT pad
            vT_sb = cpool.tile([CV, N], BF16)   # cols [mt*128,(mt+1)*128) = v[:, chunk].T
            ones_g = cpool.tile([MT, MT], BF16)  # all-ones (S-reduce stationary)
            zbias = cpool.tile([MT, 1], F32)

            # ---- loads: critical-path tensors first, bulk on gpsimd queue ----
            nc.sync.dma_start(aT_sb[:], aT[:])
            nc.sync.dma_start(xq_sb[:, :NG], xq[:, :NG])
            nc.sync.dma_start(xq_sb[:, NG:], xq[:, NG:])
            for j in range(N // NG):
                nc.sync.dma_start(xb_sb[:, ts(j, NG)], xb[:, ts(j, NG)])

            nc.gpsimd.memset(zbias[:], 0.0)
            ones_stage = cpool.tile([MT, MT], F32)
            nc.gpsimd.memset(ones_stage[:], 1.0)
            nc.gpsimd.dma_start(wvT_sb[:], wvT[:])
            for j in range(N // NG):
                nc.gpsimd.dma_start(xhb_sb[:, ts(j, NG)], xhb[:, ts(j, NG)])
            for j in range(NQ // NG):
                nc.gpsimd.dma_start(xhres_sb[:, ts(j, NG)], xh_res[:, ts(j, NG)])
            nc.vector.tensor_copy(ones_g[:], ones_stage[:])

            # ---- qk projection chunk: qk[:, j*512:] = (Wk^T Wq) @ xq chunk ----
            # aT's zero columns 64..127 write qk's zero rows; scalar engine
            # (idle at startup) does the PSUM->SBUF bf16 copy.
            def emit_qk(j):
                qk_ps = epool.tile([MT, NG], F32, tag="e", name=f"qkp_{j}")
                nc.tensor.matmul(qk_ps[:], aT_sb[:], xq_sb[:, ts(j, NG)],
                                 start=True, stop=True)
                nc.scalar.copy(qk_sb[:, ts(j, NG)], qk_ps[:])

            # ---- vT projection block j: vT[m, c] for m in [j*512,(j+1)*512) ----
            def emit_vblk(j):
                vt_ps = mpool.tile([CV, NG], F32, tag="mpsum", name=f"vtp_{j}")
                for u in range(NG // MT):
                    mt = j * (NG // MT) + u
                    nc.tensor.matmul(vt_ps[:, ts(u, MT)], xhb_sb[:, ts(mt, MT)],
                                     wvT_sb[:], start=True, stop=True)
                nc.vector.tensor_copy(vT_sb[:, ts(j, NG)], vt_ps[:])

            # ---- main flash loop (flat, software-pipelined, PAIRED) ----
            PIPE = 2          # pipeline depth in pairs
            NPAIRS_G = N_MTILES // 2
            NPT = N_GROUPS * NPAIRS_G
            HALF = NPAIRS_G // 2  # sub-chain length in pairs

            def emit_Epair(g, pp):
                e2 = epool.tile([MT, 2 * NG], F32, tag="e", name=f"e_{g}_{pp}")
                nc.tensor.matmul(e2[:, :NG], xb_sb[:, ts(2 * pp, MT)],
                                 qk_sb[:, ts(g, NG)], start=True, stop=True)
                nc.tensor.matmul(e2[:, NG:], xb_sb[:, ts(2 * pp + 1, MT)],
                                 qk_sb[:, ts(g, NG)], start=True, stop=True)
                return e2

            def emit_sreduce(s_ps, chain, first, last):
                nc.tensor.matmul(s_ps[:], ones_g[:], chain[:, :NG],
                                 start=first, stop=False)
                nc.tensor.matmul(s_ps[:], ones_g[:], chain[:, NG:],
                                 start=False, stop=last)

            def emit_epilogue(g, u_ps, s_ps, chainB, gamma):
                emit_sreduce(s_ps, chainB, first=False, last=True)
                r_sb = opool.tile([CV, NG], F32, tag="r", name=f"r_{g}")
                nc.vector.reciprocal_approx_fast(out=r_sb[:], in_=s_ps[:])
                o_sb = opool.tile([CV, NG], F32, tag="o", name=f"o_{g}")
                nc.vector.tensor_mul(o_sb[:], u_ps[:], r_sb[:])
                nc.vector.scalar_tensor_tensor(
                    out=o_sb[:], in0=o_sb[:], scalar=gamma,
                    in1=xhres_sb[:, ts(g, NG)],
                    op0=mybir.AluOpType.mult, op1=mybir.AluOpType.add)
                nc.sync.dma_start(o[:, ts(g, NG)], o_sb[:])

            # startup: qk chunk 0, then the first E-pairs immediately; the
            # remaining qk chunks and the vT blocks interleave into group 0.
            emit_qk(0)
            e_tiles = {p: emit_Epair(p // NPAIRS_G, p % NPAIRS_G)
                       for p in range(PIPE)}
            emit_qk(1)
            emit_vblk(0)
            emit_vblk(1)
            u_ps = None
            s_ps = None
            chains = None
            pending = None
            for p in range(NPT):
                g, pp = divmod(p, NPAIRS_G)
                if pp == 0:
                    u_ps = upool.tile([CV, NG], F32, tag="u", name=f"u_{g}")
                    s_ps = spool.tile([CV, NG], F32, tag="s", name=f"s_{g}")
                    chains = [sapool.tile([MT, 2 * NG], BF16, tag=f"sc{c}",
                                          name=f"sc{c}_{g}")
                              for c in range(2)]
                pt2 = ptpool.tile([MT, 2 * NG], BF16, tag="pt",
                                  name=f"pt_{g}_{pp}")
                nc.scalar.activation(pt2[:], e_tiles.pop(p)[:],
                                     mybir.ActivationFunctionType.Exp,
                                     bias=zbias[:])
                if p + PIPE < NPT:
                    gn, ppn = divmod(p + PIPE, NPAIRS_G)
                    e_tiles[p + PIPE] = emit_Epair(gn, ppn)
                if g == 0:
                    if 2 <= pp <= 7:
                        emit_vblk(pp)
                    elif pp == 8:
                        emit_qk(2)
                    elif pp == 9:
                        emit_qk(3)
                lastp = pp == NPAIRS_G - 1
                # U[c, n] += vT_tile.T @ P^T  (both halves of the pair)
                nc.tensor.matmul(u_ps[:], vT_sb[:, ts(2 * pp, MT)],
                                 pt2[:, :NG], start=(pp == 0), stop=False)
                nc.tensor.matmul(u_ps[:], vT_sb[:, ts(2 * pp + 1, MT)],
                                 pt2[:, NG:], start=False, stop=lastp)
                # S chain accumulation on DVE (bf16, full 1024-wide)
                sub = pp // HALF
                chain = chains[sub]
                if pp % HALF == 0:
                    nc.vector.tensor_copy(chain[:], pt2[:])
                else:
                    nc.vector.tensor_add(chain[:], chain[:], pt2[:])
                if pp == HALF + 2:
                    emit_sreduce(s_ps, chains[0], first=True, last=False)
                if pending is not None and (pp >= 6 or p == NPT - 1):
                    emit_epilogue(*pending)
                    pending = None
                if lastp:
                    pending = (g, u_ps, s_ps, chains[1], gamma)
            emit_epilogue(*pending)

    nc.compile()
    return nc


def kernel(x, x_h, Wq, Wk, Wv, gamma):
    global _last_results
    import ml_dtypes
    bf16 = ml_dtypes.bfloat16

    x = np.ascontiguousarray(np.asarray(x, dtype=np.float32))
    x_h = np.ascontiguousarray(np.asarray(x_h, dtype=np.float32))
    Wq = np.asarray(Wq, dtype=np.float32)
    Wk = np.asarray(Wk, dtype=np.float32)
    Wv = np.asarray(Wv, dtype=np.float32)
    gval = float(np.asarray(gamma).reshape(-1)[0])

    nc = build_bass(gval)

    # qk = (Wk^T Wq) @ xq  ->  stationary operand is (Wk^T Wq)^T = Wq^T Wk,
    # padded with zero columns so the matmul writes qk rows 64..127 as zero.
    aT = np.zeros((CQK, MT), dtype=np.float32)
    aT[:, :CQK] = Wq.T @ Wk
    aT = aT.astype(bf16)
    wvT = np.ascontiguousarray(Wv.T).astype(bf16)
    x_bf = x.astype(bf16)
    xb_pad = np.zeros((B, MT, N), dtype=bf16)
    xb_pad[:, :CQK, :] = x_bf

    in_maps = []
    for core in range(8):
        b, h = core // 2, core % 2
        sl = slice(h * NQ, (h + 1) * NQ)
        in_maps.append({
            "xb": xb_pad[b],
            "xhb": x_h[b].astype(bf16),
            "xq": np.ascontiguousarray(x_bf[b][:, sl]),
            "xh_res": np.ascontiguousarray(x_h[b][:, sl]),
            "aT": aT,
            "wvT": wvT,
        })

    res = run_bass_kernel_spmd(nc, in_maps, list(range(8)))
    _last_results = res

    out = np.empty((B, CV, N), dtype=np.float32)
    for core in range(8):
        b, h = core // 2, core % 2
        out[b][:, h * NQ:(h + 1) * NQ] = res.results[core]["o"]
    return out
